# revision 1
# baseline (speedup 1.0000x reference)
"""ARD-RBF kernel matrix on 8 Trainium2 NeuronCores.

out = variance * exp(-0.5 * (sq1[:,None] + sq2[None,:] - 2*cross))
with alpha = softmax(softplus(alpha_raw)), variance = variance_raw[0]**2,
cross = (x1*alpha) @ x2.T, sq1 = (x1*x1)@alpha, sq2 = (x2*x2)@alpha.

Strategy (rows of x1 sharded 8 ways; x2/alpha/variance replicated):
  - host ships x1.T shard [512,1024] f32 and x2.T [512,8192] bf16 — pure
    layout/precision prep; every reference FLOP runs on device.
  - unnormalized-softmax trick: with u = 1+e^alpha_raw and S = sum(u),
    every alpha-weighted sum is (1/S)*(u-weighted sum). The GEMM runs
    u-weighted bf16; 1/S rides the final Exp activation's per-partition
    f32 scale, and r1u = -0.5*sum(u*x1^2) rides its bias (both exact).
  - column term applied multiplicatively: ec2 = variance*exp(c2) is an
    f32 row, partition-broadcast on GpSimd, multiplied in by DVE.
  - ramp: constants+HAM-warmup matmuls first, latency-critical small
    loads ahead of bulk loads on the in-order sync DMA queue, [1,N] ->
    [128,N/128] scatters via PE transpose-mode matmuls (not DMA).
  - c2-prep for each 2048-column super-group is emitted one super-group
    ahead of the m-loop that consumes it.
"""

import os
import sys

import numpy as np

sys.path.insert(0, "/opt/trn_rl_repo")

import ml_dtypes

N_CORES = 8
N_ROWS, M_COLS, DIM = 8192, 8192, 512
ROWS = N_ROWS // N_CORES  # 1024 rows of x1 per core


def build_ard_rbf(tc, out, x1t, x2t, araw, vraw, rows, m_cols, dim,
                  distributed_prep=False):
    """Emit the per-core kernel. APs: out [rows, m_cols] f32,
    x1t [dim, rows] f32, x2t [dim, m_cols] bf16, araw [dim] f32,
    vraw [1] f32.

    Uses the unnormalized-softmax trick: with u = 1+e^araw (softplus'd
    softmax numerator) and S = sum(u), every alpha-weighted sum equals
    (1/S) * (u-weighted sum). The GEMM runs u-weighted; 1/S rides the
    final Exp activation's per-partition f32 scale. This keeps the
    pre-matmul dependency chain to ~5 hops.
    """
    import concourse.mybir as mybir

    nc = tc.nc
    f32 = mybir.dt.float32
    bf16 = mybir.dt.bfloat16
    AF = mybir.ActivationFunctionType

    KC = dim // 128          # contraction chunks (4)
    MT = rows // 128         # output row tiles per core (8)
    NG = m_cols // 1024      # 1024-col groups (8)

    with (
        tc.tile_pool(name="const", bufs=1) as const,
        tc.tile_pool(name="x2pool", bufs=1) as x2pool,
        tc.tile_pool(name="work", bufs=3) as work,
        tc.tile_pool(name="outp", bufs=4) as outp,
        tc.tile_pool(name="psum", bufs=3, space="PSUM") as psum,
        tc.tile_pool(name="psmall", bufs=2, space="PSUM") as psmall,
    ):
        # ---------------- constants + HAM warmup first (no deps) -----------
        id1 = const.tile([1, 1], f32)
        nc.vector.memset(id1, 1.0)
        ones128 = const.tile([1, 128], f32)
        nc.vector.memset(ones128, 1.0)
        ones512 = const.tile([1, 512], f32)
        nc.vector.memset(ones512, 1.0)
        warm_ps = psmall.tile([128, 512], f32, tag="c2ps", name="warm_ps")
        for w in range(10):
            nc.tensor.matmul(warm_ps, lhsT=ones128, rhs=ones512,
                             start=True, stop=True)

        # ---------------- loads: tiny ones first, then x2 g0, x1, x2 rest --
        # (sync/HWDGE issue is in-order; channel FIFOs mean anything queued
        # behind a bulk transfer waits for it, so the latency-critical small
        # loads go first)
        a_row = const.tile([1, dim], f32)
        nc.sync.dma_start(out=a_row, in_=araw.rearrange("(a d) -> a d", a=1))
        vr = const.tile([1, 1], f32)
        nc.sync.dma_start(out=vr, in_=vraw.rearrange("(a d) -> a d", a=1))
        x2_c = [
            x2pool.tile([128, m_cols], bf16, tag=f"x2{k}", name=f"x2_{k}")
            for k in range(KC)
        ]
        for k in range(KC):
            nc.sync.dma_start(
                out=x2_c[k][:, 0:1024], in_=x2t[k * 128 : (k + 1) * 128, 0:1024])
        x1t_c = []
        for k in range(KC):
            xt = const.tile([128, rows], f32, tag=f"x1t{k}", name=f"x1t_{k}")
            nc.sync.dma_start(out=xt, in_=x1t[k * 128 : (k + 1) * 128, :])
            x1t_c.append(xt)
        for g in range(1, NG):
            gsl = slice(g * 1024, (g + 1) * 1024)
            for k in range(KC):
                nc.sync.dma_start(
                    out=x2_c[k][:, gsl],
                    in_=x2t[k * 128 : (k + 1) * 128, gsl],
                )

        # ---------------- u = 1 + exp(araw); critical path ------------------
        # e0 [1,dim] -> [128,KC] scatter via PE transpose-mode matmuls (the
        # PE is idle during the ramp; DMA scatters cost 3-5us each on SWDGE)
        e0 = const.tile([1, dim], f32)
        sm = const.tile([1, 1], f32)
        nc.scalar.activation(e0, a_row, AF.Exp, accum_out=sm)
        ep_ps = psmall.tile([128, KC], f32, tag="c2ps", name="ep_ps")
        for k in range(KC):
            nc.tensor.transpose(
                ep_ps[:, k : k + 1], e0[:, k * 128 : (k + 1) * 128], id1)
        with tc.high_priority():
            u_p = const.tile([128, KC], f32)
            nc.vector.tensor_scalar_add(u_p, ep_ps, 1.0)
            uneg_p = const.tile([128, KC], bf16)
            nc.vector.tensor_scalar(
                uneg_p, ep_ps, -0.5, -0.5,
                op0=mybir.AluOpType.mult, op1=mybir.AluOpType.add,
            )

            # x1u = u * x1 (bf16 stationary operands for the main GEMM)
            x1a_c = []
            for k in range(KC):
                xa = const.tile([128, rows], bf16, tag=f"x1a{k}",
                                name=f"x1a_{k}")
                nc.vector.tensor_scalar_mul(xa, x1t_c[k], u_p[:, k : k + 1])
                x1a_c.append(xa)

        # ---------------- rs = 1/(dim + sum(e)); off critical path ----------
        smd = const.tile([1, 1], f32)
        nc.vector.tensor_scalar_add(smd, sm, float(dim))
        rs = const.tile([1, 1], f32)
        nc.vector.reciprocal(rs, smd)
        # rs broadcast to [128,1] via a K=1 f32 matmul (exact; PE is free)
        rs_ps = psmall.tile([128, 1], f32, tag="c2ps", name="rs_ps")
        nc.tensor.matmul(rs_ps, lhsT=ones128, rhs=rs, start=True, stop=True)
        rs128 = const.tile([128, 1], f32)
        nc.vector.tensor_copy(rs128, rs_ps)
        var = const.tile([1, 1], f32)
        nc.vector.tensor_mul(var, vr, vr)

        # ---------------- r1u = -0.5*sum(u*x1^2); scale by rs --------------
        r1_row = const.tile([1, rows], f32)
        for h in range(rows // 512):
            ps = psmall.tile([1, 512], f32, tag="c2ps")
            for k in range(KC):
                sq = work.tile([128, 512], bf16, tag="sqx1")
                src = x1t_c[k][:, h * 512 : (h + 1) * 512]
                nc.vector.tensor_mul(sq, src, src)
                nc.tensor.matmul(
                    ps, lhsT=uneg_p[:, k : k + 1], rhs=sq,
                    start=(k == 0), stop=(k == KC - 1),
                )
            nc.vector.tensor_copy(r1_row[:, h * 512 : (h + 1) * 512], ps)
        r1_ps = psmall.tile([128, MT], f32, tag="c2ps", name="r1_ps")
        for t in range(MT):
            nc.tensor.transpose(
                r1_ps[:, t : t + 1], r1_row[:, t * 128 : (t + 1) * 128], id1)
        r1_t = const.tile([128, MT], f32)
        nc.vector.tensor_scalar_mul(r1_t, r1_ps, rs128)

        # ---------------- main loop: 2048-col super-groups, m inner --------
        # c2-prep for super-group s2+1 is emitted in small pieces spread
        # across s2's m-iterations, so the in-order DVE/PE streams never
        # hit a bunched prep fence (squares at m=0..3, M=1 matmuls + exps
        # at m=4..5, variance scale + broadcast-bounce DMAs at m=6).
        NS = m_cols // 2048
        prep = {}

        def prep_piece(s2n, m):
            st = prep.setdefault(s2n, {"sq": {}, "row": None, "b": None})
            base = s2n * 2048
            if m <= 3:
                g, k0 = divmod(m, 2)
                for k in (2 * k0, 2 * k0 + 1):
                    g0 = base + g * 1024
                    sq2 = work.tile(
                        [128, 1024], bf16, tag=f"sqx2{k}", bufs=2,
                        name=f"sq2_{k}",
                    )
                    nc.vector.tensor_mul(
                        sq2, x2_c[k][:, g0 : g0 + 1024],
                        x2_c[k][:, g0 : g0 + 1024],
                    )
                    st["sq"][(g, k)] = sq2
            elif m in (4, 5):
                g = m - 4
                if st["row"] is None:
                    st["row"] = work.tile([1, 2048], f32, tag="ec2row",
                                          bufs=2, name="ec2_row")
                for h in range(2):
                    cps = psmall.tile([1, 512], f32, tag="c2ps", name="cps")
                    hs = slice(h * 512, (h + 1) * 512)
                    for k in range(KC):
                        nc.tensor.matmul(
                            cps, lhsT=uneg_p[:, k : k + 1],
                            rhs=st["sq"][(g, k)][:, hs],
                            start=(k == 0), stop=(k == KC - 1),
                        )
                    nc.scalar.activation(
                        st["row"][:, g * 1024 + h * 512 : g * 1024 + (h + 1) * 512],
                        cps, AF.Exp, scale=rs,
                    )
            elif m == 6:
                row = st["row"]
                ec2b = work.tile([128, 2048], f32, tag="ec2b", bufs=2,
                                 name="ec2b")
                for g in range(2):
                    gs = slice(g * 1024, (g + 1) * 1024)
                    nc.vector.tensor_scalar_mul(row[:, gs], row[:, gs], var)
                    # on-chip replicate; a chained DRAM-bounce DMA pair is
                    # faster but its DMA-DMA ordering proved racy on HW
                    nc.gpsimd.partition_broadcast(ec2b[:, gs], row[:, gs])
                st["b"] = ec2b

        for m in range(7):
            prep_piece(0, m)

        for s2 in range(NS):
            ssl = slice(s2 * 2048, (s2 + 1) * 2048)
            ec2b = prep.pop(s2)["b"]
            if s2 + 1 < NS and not distributed_prep:
                for mm_ in range(7):
                    prep_piece(s2 + 1, mm_)

            last = s2 == NS - 1
            for m in range(MT):
                ot = None if last else outp.tile([128, 2048], f32, tag="ot",
                                                 bufs=4, name="ot")
                for g in range(2):
                    g0 = s2 * 2048 + g * 1024
                    ps = psum.tile([128, 1024], f32, tag="mainps")
                    for h in range(2):
                        sl = slice(g0 + h * 512, g0 + (h + 1) * 512)
                        pslice = ps[:, h * 512 : (h + 1) * 512]
                        for k in range(KC):
                            nc.tensor.matmul(
                                pslice,
                                lhsT=x1a_c[k][:, m * 128 : (m + 1) * 128],
                                rhs=x2_c[k][:, sl],
                                start=(k == 0), stop=(k == KC - 1),
                            )
                    if last:
                        oth = outp.tile([128, 1024], f32, tag="oth", bufs=4,
                                        name="oth")
                        nc.scalar.activation(oth, ps, AF.Exp,
                                             bias=r1_t[:, m : m + 1],
                                             scale=rs128)
                        nc.vector.tensor_mul(
                            oth, oth, ec2b[:, g * 1024 : (g + 1) * 1024])
                        nc.sync.dma_start(
                            out=out[m * 128 : (m + 1) * 128, g0 : g0 + 1024],
                            in_=oth)
                    else:
                        nc.scalar.activation(
                            ot[:, g * 1024 : (g + 1) * 1024], ps, AF.Exp,
                            bias=r1_t[:, m : m + 1], scale=rs128,
                        )
                if not last:
                    nc.vector.tensor_mul(ot, ot, ec2b)
                    nc.sync.dma_start(
                        out=out[m * 128 : (m + 1) * 128, ssl], in_=ot)
                if s2 + 1 < NS and distributed_prep and m < 7:
                    prep_piece(s2 + 1, m)


_CACHE = {}


def _get_compiled():
    if "nc" in _CACHE:
        return _CACHE["nc"]
    import concourse.mybir as mybir
    import concourse.tile as tile
    from concourse import bacc

    f32 = mybir.dt.float32
    bf16 = mybir.dt.bfloat16
    nc = bacc.Bacc("TRN2", target_bir_lowering=False, debug=False,
                   enable_asserts=False)
    x1t = nc.dram_tensor("x1t", [DIM, ROWS], f32, kind="ExternalInput").ap()
    x2t = nc.dram_tensor("x2t", [DIM, M_COLS], bf16, kind="ExternalInput").ap()
    araw = nc.dram_tensor("alpha_raw", [DIM], f32, kind="ExternalInput").ap()
    vraw = nc.dram_tensor("variance_raw", [1], f32, kind="ExternalInput").ap()
    out = nc.dram_tensor("out", [ROWS, M_COLS], f32, kind="ExternalOutput").ap()

    with tile.TileContext(nc) as tc:
        build_ard_rbf(tc, out, x1t, x2t, araw, vraw, ROWS, M_COLS, DIM)
    nc.compile()
    _CACHE["nc"] = nc
    return nc


def kernel(x1, x2, alpha_raw, variance_raw):
    from concourse import bass_utils

    x1 = np.asarray(x1, dtype=np.float32)
    x2 = np.asarray(x2, dtype=np.float32)
    alpha_raw = np.ascontiguousarray(np.asarray(alpha_raw, dtype=np.float32))
    variance_raw = np.ascontiguousarray(
        np.asarray(variance_raw, dtype=np.float32))

    x1t_full = np.ascontiguousarray(x1.T)                      # [512, 8192] f32
    x2t_full = np.ascontiguousarray(x2.T).astype(ml_dtypes.bfloat16)

    nc = _get_compiled()
    in_maps = []
    for c in range(N_CORES):
        in_maps.append({
            "x1t": np.ascontiguousarray(x1t_full[:, c * ROWS : (c + 1) * ROWS]),
            "x2t": x2t_full,
            "alpha_raw": alpha_raw,
            "variance_raw": variance_raw,
        })
    res = bass_utils.run_bass_kernel_spmd(
        nc, in_maps, core_ids=list(range(N_CORES)),
        trace=bool(int(os.environ.get("ARD_TRACE", "0"))),
        tmpdir=os.environ.get("ARD_TMPDIR"),
    )
    _CACHE["last_results"] = res
    out = np.concatenate([res.results[c]["out"] for c in range(N_CORES)], axis=0)
    return out


if __name__ == "__main__":
    rng = np.random.default_rng(0)
    ins = {
        "x1": rng.standard_normal((N_ROWS, DIM), dtype=np.float32),
        "x2": rng.standard_normal((M_COLS, DIM), dtype=np.float32),
        "alpha_raw": rng.standard_normal((DIM,), dtype=np.float32),
        "variance_raw": rng.random((1,), dtype=np.float32),
    }
    o = kernel(**ins)
    print(o.shape, o.dtype)



# revision 3
# speedup vs baseline: 1.2651x; 1.2651x over previous
"""ARD-RBF kernel matrix on 8 Trainium2 NeuronCores (fp8 DoubleRow version).

out = variance * exp(-0.5 * (sq1[:,None] + sq2[None,:] - 2*cross))
with alpha = softmax(softplus(alpha_raw)), variance = variance_raw[0]**2,
cross = (x1*alpha) @ x2.T, sq1 = (x1*x1)@alpha, sq2 = (x2*x2)@alpha.

Strategy (rows of x1 sharded 8 ways; x2/alpha/variance replicated):
  - host ships x1.T shard [512,1024] bf16 and x2.T [512,8192] fp8e4m3 —
    pure layout/precision prep; every reference FLOP runs on device.
  - unnormalized-softmax trick: with u = 1+e^alpha_raw and S = sum(u),
    every alpha-weighted sum is (1/S)*(u-weighted sum).
  - main GEMM in fp8 DoubleRow perf mode: x1a = (u*x1)/4 quantized fp8,
    x2 fp8; K=512 contracted as 2 DoubleRow pairs (2 chunks of 128 per
    instruction).  4/S rides the final Exp activation's per-partition
    f32 scale; r1u = -0.5*sum(u*x1^2) (bf16 path, exact u) rides bias.
  - column term ec2 = variance*exp(-0.5*rs*r2u) computed from fp8
    squares of x2, applied multiplicatively by DVE in bf16 (2x/4x DVE).
  - output written bf16 (halves write traffic); host upcasts to f32.
  - ramp: constants+HAM-warmup matmuls first, latency-critical small
    loads ahead of bulk loads on the in-order sync DMA queue, [1,N] ->
    [128,N/128] scatters via PE transpose-mode matmuls (not DMA).
"""

import os
import sys

import numpy as np

sys.path.insert(0, "/opt/trn_rl_repo")

import ml_dtypes

N_CORES = 8
N_ROWS, M_COLS, DIM = 8192, 8192, 512
ROWS = N_ROWS // N_CORES  # 1024 rows of x1 per core
S1 = 4.0                  # x1a fp8 pre-scale (undone via Exp scale)


def build_ard_rbf(tc, out, x1t, x2t, araw, vraw, rows, m_cols, dim):
    """Emit the per-core kernel. APs: out [rows, m_cols] bf16,
    x1t [dim, rows] bf16, x2t [dim, m_cols] fp8e4, araw [dim] f32,
    vraw [1] f32."""
    import concourse.mybir as mybir

    nc = tc.nc
    f32 = mybir.dt.float32
    bf16 = mybir.dt.bfloat16
    f8 = mybir.dt.float8e4
    AF = mybir.ActivationFunctionType
    DR = mybir.MatmulPerfMode.DoubleRow

    KC = dim // 128          # contraction chunks (4)
    KP = KC // 2             # DoubleRow chunk pairs (2)
    MT = rows // 128         # output row tiles per core (8)

    with (
        tc.tile_pool(name="const", bufs=1) as const,
        tc.tile_pool(name="x2pool", bufs=1) as x2pool,
        tc.tile_pool(name="work", bufs=2) as work,
        tc.tile_pool(name="outp", bufs=4) as outp,
        tc.tile_pool(name="psum", bufs=3, space="PSUM") as psum,
        tc.tile_pool(name="psmall", bufs=2, space="PSUM") as psmall,
    ):
        # ---------------- constants + HAM warmup first (no deps) -----------
        id1 = const.tile([1, 1], f32)
        nc.vector.memset(id1, 1.0)
        ones128 = const.tile([1, 128], f32)
        nc.vector.memset(ones128, 1.0)
        ones512 = const.tile([1, 512], f32)
        nc.vector.memset(ones512, 1.0)
        warm_ps = psmall.tile([128, 512], f32, tag="c2ps", name="warm_ps")
        for w in range(10):
            nc.tensor.matmul(warm_ps, lhsT=ones128, rhs=ones512,
                             start=True, stop=True)

        # ---------------- loads: tiny ones first, then x2 s0, x1, x2 rest --
        a_row = const.tile([1, dim], f32)
        nc.sync.dma_start(out=a_row, in_=araw.rearrange("(a d) -> a d", a=1))
        vr = const.tile([1, 1], f32)
        nc.sync.dma_start(out=vr, in_=vraw.rearrange("(a d) -> a d", a=1))
        # x2 fp8 in DoubleRow pair layout: x2f[kk][:, j, :] = chunk 2kk+j
        x2f = [
            x2pool.tile([128, 2, m_cols], f8, tag=f"x2f{kk}", name=f"x2f{kk}")
            for kk in range(KP)
        ]

        def load_x2(k, gsl):
            kk, j = divmod(k, 2)
            nc.sync.dma_start(
                out=x2f[kk][:, j : j + 1, gsl],
                in_=x2t[k * 128 : (k + 1) * 128, gsl],
            )

        for g in range(2):           # first supergroup's columns
            for k in range(KC):
                load_x2(k, slice(g * 1024, (g + 1) * 1024))
        x1t_c = []
        for k in range(KC):
            xt = const.tile([128, rows], bf16, tag=f"x1t{k}", name=f"x1t_{k}")
            nc.sync.dma_start(out=xt, in_=x1t[k * 128 : (k + 1) * 128, :])
            x1t_c.append(xt)
        for g in range(2, m_cols // 1024):
            for k in range(KC):
                load_x2(k, slice(g * 1024, (g + 1) * 1024))

        # ---------------- u = 1 + exp(araw); critical path ------------------
        e0 = const.tile([1, dim], f32)
        sm = const.tile([1, 1], f32)
        nc.scalar.activation(e0, a_row, AF.Exp, accum_out=sm)
        ep_ps = psmall.tile([128, KC], f32, tag="c2ps", name="ep_ps")
        for k in range(KC):
            nc.tensor.transpose(
                ep_ps[:, k : k + 1], e0[:, k * 128 : (k + 1) * 128], id1)
        with tc.high_priority():
            # u4_p = (1+e)/S1 per chunk column; uneg8 = -0.5*(1+e) fp8
            u4_p = const.tile([128, KC], f32)
            nc.vector.tensor_scalar(
                u4_p, ep_ps, 1.0 / S1, 1.0 / S1,
                op0=mybir.AluOpType.mult, op1=mybir.AluOpType.add,
            )
            uneg8 = const.tile([128, KC, 1], f8)
            nc.vector.tensor_scalar(
                uneg8, ep_ps, -0.5, -0.5,
                op0=mybir.AluOpType.mult, op1=mybir.AluOpType.add,
            )
            unegb = const.tile([128, KC], bf16)
            nc.vector.tensor_scalar(
                unegb, ep_ps, -0.5, -0.5,
                op0=mybir.AluOpType.mult, op1=mybir.AluOpType.add,
            )

            # x1a = (u/S1) * x1, fp8, DoubleRow pair layout
            x1f = [
                const.tile([128, 2, rows], f8, tag=f"x1f{kk}", name=f"x1f{kk}")
                for kk in range(KP)
            ]
            for k in range(KC):
                kk, j = divmod(k, 2)
                nc.vector.tensor_scalar_mul(
                    x1f[kk][:, j : j + 1, :], x1t_c[k], u4_p[:, k : k + 1])

        # ---------------- rs = 1/(dim + sum(e)); off critical path ----------
        smd = const.tile([1, 1], f32)
        nc.vector.tensor_scalar_add(smd, sm, float(dim))
        rs = const.tile([1, 1], f32)
        nc.vector.reciprocal(rs, smd)
        # rs_pair = [rs, S1*rs]; broadcast to [128,2] via K=1 f32 matmul
        rs_pair = const.tile([1, 2], f32)
        nc.vector.tensor_copy(rs_pair[:, 0:1], rs)
        nc.vector.tensor_scalar_mul(rs_pair[:, 1:2], rs, S1)
        rs_ps = psmall.tile([128, 2], f32, tag="c2ps", name="rs_ps")
        nc.tensor.matmul(rs_ps, lhsT=ones128, rhs=rs_pair, start=True,
                         stop=True)
        rs_bc = const.tile([128, 2], f32)
        nc.vector.tensor_copy(rs_bc, rs_ps)
        var = const.tile([1, 1], f32)
        nc.vector.tensor_mul(var, vr, vr)

        # ---------------- r1u = -0.5*sum(u*x1^2); scale by rs --------------
        r1_row = const.tile([1, rows], f32)
        for h in range(rows // 512):
            ps = psmall.tile([1, 512], f32, tag="c2ps")
            for k in range(KC):
                sq = work.tile([128, 512], bf16, tag="sqx1")
                src = x1t_c[k][:, h * 512 : (h + 1) * 512]
                nc.vector.tensor_mul(sq, src, src)
                nc.tensor.matmul(
                    ps, lhsT=unegb[:, k : k + 1], rhs=sq,
                    start=(k == 0), stop=(k == KC - 1),
                )
            nc.vector.tensor_copy(r1_row[:, h * 512 : (h + 1) * 512], ps)
        r1_ps = psmall.tile([128, MT], f32, tag="c2ps", name="r1_ps")
        for t in range(MT):
            nc.tensor.transpose(
                r1_ps[:, t : t + 1], r1_row[:, t * 128 : (t + 1) * 128], id1)
        r1_t = const.tile([128, MT], f32)
        nc.vector.tensor_scalar_mul(r1_t, r1_ps, rs_bc[:, 0:1])

        # ---------------- main loop: 2048-col super-groups, m inner --------
        # c2-prep for super-group s2+1 is emitted in pieces ahead of the
        # m-loop that consumes it: fp8 squares (m=0..3), DoubleRow M=1
        # matmuls + exps (m=4..5), variance scale + gpsimd broadcast (m=6).
        NS = m_cols // 2048
        prep = {}

        def prep_piece(s2n, m):
            st = prep.setdefault(s2n, {"sq": {}, "row": None, "b": None})
            base = s2n * 2048
            if m <= 3:
                kk, g = divmod(m, 2)
                if kk == 0 and g == 0:
                    for kk2 in range(KP):
                        st["sq"][kk2] = work.tile(
                            [128, 2, 2048], f8, tag=f"sqx2{kk2}", bufs=2,
                            name=f"sq2_{kk2}",
                        )
                gsl = slice(base + g * 1024, base + (g + 1) * 1024)
                dsl = slice(g * 1024, (g + 1) * 1024)
                nc.vector.tensor_mul(
                    st["sq"][kk][:, :, dsl], x2f[kk][:, :, gsl],
                    x2f[kk][:, :, gsl],
                )
            elif m in (4, 5):
                if st["row"] is None:
                    st["row"] = work.tile([1, 2048], f32, tag="ec2row",
                                          bufs=2, name="ec2_row")
                for h in range(2 * (m - 4), 2 * (m - 4) + 2):
                    cps = psmall.tile([1, 512], f32, tag="c2ps", name="cps")
                    hs = slice(h * 512, (h + 1) * 512)
                    # plain fp8 per-chunk: M=1 DoubleRow fails the ISA's
                    # s3_lw_dual_fp8_restrictions check (and would be slower
                    # anyway -- DR only pays off for weight free dim >= 128)
                    for kk in range(KP):
                        for j in range(2):
                            nc.tensor.matmul(
                                cps,
                                lhsT=uneg8[:, 2 * kk + j : 2 * kk + j + 1, :],
                                rhs=st["sq"][kk][:, j : j + 1, hs],
                                start=(kk == 0 and j == 0),
                                stop=(kk == KP - 1 and j == 1),
                            )
                    nc.scalar.activation(
                        st["row"][:, hs], cps, AF.Exp, scale=rs,
                    )
            elif m == 6:
                rowb = work.tile([1, 2048], bf16, tag="ec2rowb", bufs=2,
                                 name="ec2_rowb")
                nc.vector.tensor_scalar_mul(rowb, st["row"], var)
                ec2b = work.tile([128, 2048], bf16, tag="ec2b", bufs=2,
                                 name="ec2b")
                for g in range(2):
                    gs = slice(g * 1024, (g + 1) * 1024)
                    nc.gpsimd.partition_broadcast(ec2b[:, gs], rowb[:, gs])
                st["b"] = ec2b

        for m in range(7):
            prep_piece(0, m)

        for s2 in range(NS):
            ssl = slice(s2 * 2048, (s2 + 1) * 2048)
            ec2b = prep.pop(s2)["b"]
            if s2 + 1 < NS:
                for mm_ in range(7):
                    prep_piece(s2 + 1, mm_)

            for m in range(MT):
                ot = outp.tile([128, 2048], bf16, tag="ot", bufs=4, name="ot")
                for g in range(2):
                    g0 = s2 * 2048 + g * 1024
                    ps = psum.tile([128, 1024], f32, tag="mainps")
                    for h in range(2):
                        sl = slice(g0 + h * 512, g0 + (h + 1) * 512)
                        pslice = ps[:, h * 512 : (h + 1) * 512]
                        for kk in range(KP):
                            nc.tensor.matmul(
                                pslice,
                                lhsT=x1f[kk][:, :, m * 128 : (m + 1) * 128],
                                rhs=x2f[kk][:, :, sl],
                                start=(kk == 0), stop=(kk == KP - 1),
                                perf_mode=DR,
                            )
                    nc.scalar.activation(
                        ot[:, g * 1024 : (g + 1) * 1024], ps, AF.Exp,
                        bias=r1_t[:, m : m + 1], scale=rs_bc[:, 1:2],
                    )
                nc.vector.tensor_mul(ot, ot, ec2b)
                nc.sync.dma_start(
                    out=out[m * 128 : (m + 1) * 128, ssl], in_=ot)


_CACHE = {}


def _get_compiled():
    if "nc" in _CACHE:
        return _CACHE["nc"]
    import concourse.mybir as mybir
    import concourse.tile as tile
    from concourse import bacc

    f32 = mybir.dt.float32
    bf16 = mybir.dt.bfloat16
    f8 = mybir.dt.float8e4
    nc = bacc.Bacc("TRN2", target_bir_lowering=False, debug=False,
                   enable_asserts=False)
    x1t = nc.dram_tensor("x1t", [DIM, ROWS], bf16, kind="ExternalInput").ap()
    x2t = nc.dram_tensor("x2t", [DIM, M_COLS], f8, kind="ExternalInput").ap()
    araw = nc.dram_tensor("alpha_raw", [DIM], f32, kind="ExternalInput").ap()
    vraw = nc.dram_tensor("variance_raw", [1], f32, kind="ExternalInput").ap()
    out = nc.dram_tensor("out", [ROWS, M_COLS], bf16,
                         kind="ExternalOutput").ap()

    with tile.TileContext(nc) as tc:
        build_ard_rbf(tc, out, x1t, x2t, araw, vraw, ROWS, M_COLS, DIM)
    nc.compile()
    _CACHE["nc"] = nc
    return nc


def kernel(x1, x2, alpha_raw, variance_raw):
    from concourse import bass_utils

    x1 = np.asarray(x1, dtype=np.float32)
    x2 = np.asarray(x2, dtype=np.float32)
    alpha_raw = np.ascontiguousarray(np.asarray(alpha_raw, dtype=np.float32))
    variance_raw = np.ascontiguousarray(
        np.asarray(variance_raw, dtype=np.float32))

    x1t_full = np.ascontiguousarray(x1.T).astype(ml_dtypes.bfloat16)
    x2t_full = np.ascontiguousarray(x2.T).astype(ml_dtypes.float8_e4m3)

    nc = _get_compiled()
    in_maps = []
    for c in range(N_CORES):
        in_maps.append({
            "x1t": np.ascontiguousarray(x1t_full[:, c * ROWS : (c + 1) * ROWS]),
            "x2t": x2t_full,
            "alpha_raw": alpha_raw,
            "variance_raw": variance_raw,
        })
    res = bass_utils.run_bass_kernel_spmd(
        nc, in_maps, core_ids=list(range(N_CORES)),
        trace=bool(int(os.environ.get("ARD_TRACE", "0"))),
        tmpdir=os.environ.get("ARD_TMPDIR"),
    )
    _CACHE["last_results"] = res
    out = np.concatenate(
        [res.results[c]["out"] for c in range(N_CORES)], axis=0)
    return out.astype(np.float32)


if __name__ == "__main__":
    rng = np.random.default_rng(0)
    ins = {
        "x1": rng.standard_normal((N_ROWS, DIM), dtype=np.float32),
        "x2": rng.standard_normal((M_COLS, DIM), dtype=np.float32),
        "alpha_raw": rng.standard_normal((DIM,), dtype=np.float32),
        "variance_raw": rng.random((1,), dtype=np.float32),
    }
    o = kernel(**ins)
    print(o.shape, o.dtype)
